# revision 1
# baseline (speedup 1.0000x reference)
"""GQA attention kernel for 8 TRN2 NeuronCores.

Problem: B=2, T=2048, D=2048, H=16 q-heads, KV=4 kv-heads, HD=128, RoPE,
non-causal softmax, out projection. f32 reference.

Sharding: 8 cores = 2 batches x 4 kv-groups. Core c handles batch c//4 and
kv-group c%4 (4 q heads + 1 kv head). Each core computes a partial output
x @ wq_g -> attention -> (heads g) @ wo_g^T: full [T, D] partial summed on
host over the 4 groups of each batch (tensor-parallel unshard).

On-device layout: everything transposed ([hd, t], hd=128=partition dim).
 - host feeds xT, wqT, wkT, wvT (d-on-partition chunks) so projections are
   plain lhsT.T @ rhs matmuls with K=d contraction, fp32r (full PE rate).
 - scores computed transposed: ST[s, t] = k^T q per s-chunk; softmax over s
   (partitions) uses exp on ACT + bf16 chunk-adds on DVE + a ones-matmul
   partition-reduce-broadcast on PE; normalization folded into the OT evac.
 - PV: OT[hd, t] += v_nat[s, hd]^T expST[s, t] per s-chunk (bf16).
 - out projection: out[t, d] = sum_h OTn_h[j, t]^T wogT[j, d] (bf16).
"""
import os
import sys

for _p in ("/opt/trn_rl_repo", "/root/.axon_site/_ro/trn_rl_repo"):
    if os.path.isdir(_p) and _p not in sys.path:
        sys.path.append(_p)

import numpy as np
import ml_dtypes

import concourse.bass as bass
import concourse.tile as tile
from concourse.tile import add_dep_helper
from concourse import bacc, mybir
from concourse import bass_utils
from concourse.bass_utils import run_bass_kernel_spmd

# If a caller enables tracing (BASS_TRACE=1), artifact upload may have no
# bucket access in this container; fall back to the local dir.
_orig_upload = bass_utils.upload_artifacts


def _safe_upload(tmpdir):
    try:
        return _orig_upload(tmpdir)
    except Exception:
        return tmpdir


bass_utils.upload_artifacts = _safe_upload

B, T, D = 2, 2048, 2048
H, KV, HD = 16, 4, 128
NR = H // KV  # 4 q heads per kv group
NCORES = 8
ROPE_BASE = 10000.0
SCALE = float(HD) ** -0.5

F32R = mybir.dt.float32r
F32 = mybir.dt.float32
BF16 = mybir.dt.bfloat16

_cache = {}


def _build_nc():
    nc = bacc.Bacc("TRN2", target_bir_lowering=False, debug=False,
                   num_devices=NCORES)

    xt_e = nc.dram_tensor("xt", [128, 16, T], F32R, kind="ExternalInput").ap()
    wqt_e = [nc.dram_tensor(f"wqt{j}", [128, 16, HD], F32R,
                            kind="ExternalInput").ap() for j in range(NR)]
    wkt_e = nc.dram_tensor("wkt", [128, 16, HD], F32R, kind="ExternalInput").ap()
    wvt_e = nc.dram_tensor("wvt", [128, 16, HD], F32R, kind="ExternalInput").ap()
    wot_e = nc.dram_tensor("wot", [128, NR, D], BF16, kind="ExternalInput").ap()
    cos_e = nc.dram_tensor("cosa", [128, T], F32R, kind="ExternalInput").ap()
    sin_e = nc.dram_tensor("sina", [128, T], F32R, kind="ExternalInput").ap()
    rotm_e = nc.dram_tensor("rotm", [128, 128], F32R, kind="ExternalInput").ap()
    ident_e = nc.dram_tensor("ident", [128, 128], F32R, kind="ExternalInput").ap()
    ones_e = nc.dram_tensor("ones", [128, 128], F32R, kind="ExternalInput").ap()
    out_e = nc.dram_tensor("out", [T, D], F32, kind="ExternalOutput").ap()

    with tile.TileContext(nc) as tc:
        import contextlib
        with contextlib.ExitStack() as ctx:
            consts = ctx.enter_context(tc.tile_pool(name="consts", bufs=1))
            weights = ctx.enter_context(tc.tile_pool(name="weights", bufs=1))
            acts = ctx.enter_context(tc.tile_pool(name="acts", bufs=1))

            cos_sb = consts.tile([128, T], F32R, tag="cos")
            sin_sb = consts.tile([128, T], F32R, tag="sin")
            rotm_sb = consts.tile([128, 128], F32R, tag="rotm")
            ident_sb = consts.tile([128, 128], F32R, tag="ident")
            ones_sb = consts.tile([128, 128], F32R, tag="ones")
            wkt_sb = weights.tile([128, 16, HD], F32R, tag="wkt")
            wvt_sb = weights.tile([128, 16, HD], F32R, tag="wvt")
            wqt_sb = [weights.tile([128, 16, HD], F32R, tag=f"wqt{j}",
                                   name=f"wqt{j}_sb") for j in range(NR)]
            wot_sb = weights.tile([128, NR, D], BF16, tag="wot")
            # DMA ordering: sync queue feeds the PE-critical path (wkt then
            # xt tiles); scalar queue gets everything else, smallest/most
            # urgent first, wqt split per q-head column so q-proj j starts
            # as soon as its slice lands.
            # sync queue: ONLY the PE-critical path (wkt then xt tiles).
            # scalar queue: small consts. gpsimd SWDGE: bulk weights.
            nc.sync.dma_start(out=wkt_sb[:, :4, :], in_=wkt_e[:, :4, :])
            nc.sync.dma_start(out=wkt_sb[:, 4:, :], in_=wkt_e[:, 4:, :])
            nc.scalar.dma_start(out=rotm_sb, in_=rotm_e)
            nc.scalar.dma_start(out=ident_sb, in_=ident_e)
            nc.scalar.dma_start(out=ones_sb, in_=ones_e)
            dc0 = nc.scalar.dma_start(out=cos_sb[:, :512], in_=cos_e[:, :512])
            ds0 = nc.scalar.dma_start(out=sin_sb[:, :512], in_=sin_e[:, :512])
            dwv = nc.gpsimd.dma_start(out=wvt_sb, in_=wvt_e)
            gated_dmas = [(dc0.ins, 1), (ds0.ins, 1)]
            for j in range(NR):
                dq = nc.gpsimd.dma_start(out=wqt_sb[j], in_=wqt_e[j])
                gated_dmas.append((dq.ins, max(0, 3 * j - 1)))
            dcr = nc.scalar.dma_start(out=cos_sb[:, 512:], in_=cos_e[:, 512:])
            gated_dmas.append((dcr.ins, 12))
            dsr = nc.scalar.dma_start(out=sin_sb[:, 512:], in_=sin_e[:, 512:])
            gated_dmas.append((dsr.ins, 12))

            qtr = [acts.tile([128, T], F32R, tag=f"qtr{j}", name=f"qtr{j}") for j in range(NR)]
            ktr = acts.tile([128, T], F32R, tag="ktr")
            v_sb = acts.tile([128, 16, HD], F32R, tag="vsb")  # v natural, s-chunked

            # ---------------- Phase 1: projections + RoPE + v transpose ----
            with tc.tile_pool(name="xt", bufs=8) as xt_pool, \
                 tc.tile_pool(name="rope", bufs=2) as rope_pool, \
                 tc.tile_pool(name="p1ps", bufs=1, space="PSUM") as p1ps, \
                 tc.tile_pool(name="rotps", bufs=2, space="PSUM") as rotps:
                anchors = []  # tt0 k-proj matmul instructions
                for tt in range(4):
                    tsl = slice(tt * 512, (tt + 1) * 512)
                    xq = []
                    for i in range(4):
                        xti = xt_pool.tile([128, 4, 512], F32R, tag="xt")
                        lo = i * 4
                        if tt == 0 and i == 0:
                            # split so the first k matmul waits on 256KB only
                            nc.sync.dma_start(out=xti[:, 0:1, :],
                                              in_=xt_e[:, 0:1, tsl])
                            nc.scalar.dma_start(out=xti[:, 1:4, :],
                                                in_=xt_e[:, 1:4, tsl])
                        else:
                            # stripe across both HWDGE queues for 2x delivery
                            nc.sync.dma_start(out=xti[:, 0:2, :],
                                              in_=xt_e[:, lo:lo + 2, tsl])
                            nc.scalar.dma_start(out=xti[:, 2:4, :],
                                                in_=xt_e[:, lo + 2:lo + 4, tsl])
                        xq.append(xti)
                    qps = [p1ps.tile([128, 512], F32, tag=f"qps{j}", name=f"qps{j}_{tt}") for j in range(NR)]
                    kps = p1ps.tile([128, 512], F32, tag="kps")
                    vps = p1ps.tile([128, 512], F32, tag="vps")
                    for i in range(4):
                        for dc in range(4):
                            g = i * 4 + dc
                            mk = nc.tensor.matmul(kps, wkt_sb[:, g, :],
                                                  xq[i][:, dc, :],
                                                  start=(g == 0), stop=(g == 15))
                            if tt == 0:
                                anchors.append(mk.ins)
                    for i in range(4):
                        for dc in range(4):
                            g = i * 4 + dc
                            nc.tensor.matmul(vps, wvt_sb[:, g, :], xq[i][:, dc, :],
                                             start=(g == 0), stop=(g == 15))
                    for j in range(NR):
                        for i in range(4):
                            for dc in range(4):
                                g = i * 4 + dc
                                nc.tensor.matmul(
                                    qps[j], wqt_sb[j][:, g, :],
                                    xq[i][:, dc, :], start=(g == 0), stop=(g == 15))

                    # RoPE: dst = src*cos + rotate_half(src)*sin, entirely on
                    # DVE via partition-shifted PSUM reads (sign of the lower
                    # half folded into the host sin table).
                    def rope(src, dst):
                        t1 = rope_pool.tile([128, 512], F32R, tag="t1", name="t1")
                        nc.vector.tensor_mul(t1, src, cos_sb[:, tsl])
                        t2 = rope_pool.tile([128, 512], F32R, tag="t2", name="t2")
                        nc.vector.tensor_mul(t2[0:64, :], src[64:128, :],
                                             sin_sb[0:64, tsl])
                        nc.vector.tensor_mul(t2[64:128, :], src[0:64, :],
                                             sin_sb[64:128, tsl])
                        nc.gpsimd.tensor_add(dst, t1, t2)

                    rope(kps, ktr[:, tsl])
                    # v: copy vT psum -> sbuf bf16, PE-transpose 128-blocks
                    vt_sb = rope_pool.tile([128, 512], F32R, tag="vt")
                    nc.scalar.copy(vt_sb, vps)
                    for vb in range(4):
                        tr_ps = rotps.tile([128, 128], F32R, tag="rot")
                        nc.tensor.transpose(tr_ps, vt_sb[:, vb * 128:(vb + 1) * 128],
                                            ident_sb)
                        nc.vector.tensor_copy(v_sb[:, tt * 4 + vb, :], tr_ps)
                    for j in range(NR):
                        rope(qps[j], qtr[j][:, tsl])

                for dins, aidx in gated_dmas:
                    add_dep_helper(dins, anchors[min(aidx, 15)],
                                   reason="gate bulk dma behind startup")

            # ---------------- Phase 2+3: attention + out projection --------
            dwot = nc.gpsimd.dma_start(out=wot_sb, in_=wot_e)
            add_dep_helper(dwot.ins, anchors[15], reason="gate wot dma")
            with tc.tile_pool(name="p2sb", bufs=4) as p2sb, \
                 tc.tile_pool(name="dens", bufs=3) as dens, \
                 tc.tile_pool(name="otn", bufs=2) as otnp, \
                 tc.tile_pool(name="ostg", bufs=4) as ostg, \
                 tc.tile_pool(name="stps", bufs=2, space="PSUM") as stps, \
                 tc.tile_pool(name="otps", bufs=2, space="PSUM") as otps, \
                 tc.tile_pool(name="outps", bufs=2, space="PSUM") as outps:
                pending = [None]    # deferred softmax epilogue of previous head
                pend_out = []       # deferred out-projection pieces (prev tt)

                def flush_epilogue():
                    if pending[0] is not None:
                        pending[0]()
                        pending[0] = None

                def out_piece(tt, tkc, otn_t, dts):
                    # half a t-chunk of the out projection: 8 matmuls + evacs
                    rows = slice(tt * 512 + tkc * 128, tt * 512 + (tkc + 1) * 128)
                    for dt in dts:
                        o_ps = outps.tile([128, 512], F32, tag="ops",
                                          name=f"o_ps_{tt}_{tkc}_{dt}")
                        for hh in range(NR):
                            nc.tensor.matmul(
                                o_ps, otn_t[:, hh, tkc * 128:(tkc + 1) * 128],
                                wot_sb[:, hh, dt * 512:(dt + 1) * 512],
                                start=(hh == 0), stop=(hh == NR - 1))
                        o_sb = ostg.tile([128, 512], F32, tag="ostg",
                                         name=f"o_sb_{tt}_{tkc}_{dt}")
                        if dt % 2 == 0:
                            nc.vector.tensor_copy(o_sb, o_ps)
                        else:
                            nc.scalar.copy(o_sb, o_ps)
                        nc.sync.dma_start(
                            out=out_e[rows, dt * 512:(dt + 1) * 512], in_=o_sb)

                for tt in range(4):
                    tsl = slice(tt * 512, (tt + 1) * 512)
                    otn_t = otnp.tile([128, NR, 512], BF16, tag="otn")
                    for h in range(NR):
                        ot_ps = otps.tile([128, 512], F32, tag="ot",
                                          name=f"ot_{tt}_{h}")
                        den = dens.tile([128, 2, 512], F32R, tag="den",
                                        name=f"den_{tt}_{h}")
                        exs = {}
                        # one-deep software pipeline: ST(sg) runs one step
                        # ahead of PV(sg) so PE never waits on the exp
                        for step in range(9):
                            if step < 8:
                                st_ps = stps.tile([128, 2, 512], F32, tag="st",
                                                  name=f"st_{tt}_{h}_{step}")
                                for half in range(2):
                                    sc = step * 2 + half
                                    nc.tensor.matmul(
                                        st_ps[:, half, :],
                                        ktr[:, sc * 128:(sc + 1) * 128],
                                        qtr[h][:, tsl], start=True, stop=True)
                                ex = p2sb.tile([128, 2, 512], F32R, tag="exp",
                                               name=f"ex_{tt}_{h}_{step}")
                                nc.scalar.activation(
                                    ex, st_ps, mybir.ActivationFunctionType.Exp,
                                    scale=SCALE)
                                exs[step] = ex
                                if step == 0:
                                    nc.vector.tensor_copy(den, ex)
                                else:
                                    nc.vector.tensor_add(den, den, ex)
                            if step >= 1:
                                sg = step - 1
                                for half in range(2):
                                    sc = sg * 2 + half
                                    nc.tensor.matmul(ot_ps, v_sb[:, sc, :],
                                                     exs[sg][:, half, :],
                                                     start=(sc == 0),
                                                     stop=(sc == 15))
                                if sg > 1:
                                    exs.pop(sg - 2, None)
                            if step == 2:
                                # previous head's epilogue: overlaps this
                                # head's score stream
                                flush_epilogue()
                            if step in (3, 5) and pend_out:
                                # a piece of the previous t-tile's out
                                # projection as PE filler
                                pend_out.pop(0)()

                        def epilogue(ot_ps=ot_ps, den=den, h=h,
                                     otn_t=otn_t, tt=tt):
                            # partition-reduce+broadcast denominator on PE,
                            # both halves accumulated into one PSUM bank
                            bc_ps = outps.tile([128, 512], F32, tag="ops",
                                               name=f"bc_{tt}_{h}")
                            nc.tensor.matmul(bc_ps, ones_sb, den[:, 0, :],
                                             start=True, stop=False)
                            nc.tensor.matmul(bc_ps, ones_sb, den[:, 1, :],
                                             start=False, stop=True)
                            rden = dens.tile([128, 512], F32, tag="rden",
                                             name=f"rden_{tt}_{h}")
                            nc.vector.reciprocal_approx_fast(rden, bc_ps)
                            nc.vector.tensor_tensor(out=otn_t[:, h, :], in0=ot_ps,
                                                    in1=rden,
                                                    op=mybir.AluOpType.mult)
                        pending[0] = epilogue

                    flush_epilogue()
                    pend_out = [
                        (lambda tt=tt, tkc=tkc, otn_t=otn_t, dts=dts:
                         out_piece(tt, tkc, otn_t, dts))
                        for tkc in range(4) for dts in ((0, 1), (2, 3))]
                # final t-tile's out projection
                for p in pend_out:
                    p()
    nc.compile()
    return nc


def _get_nc():
    if "nc" not in _cache:
        _cache["nc"] = _build_nc()
    return _cache["nc"]


def _host_consts():
    if "consts" in _cache:
        return _cache["consts"]
    inv = 1.0 / (ROPE_BASE ** (np.arange(0, HD, 2, dtype=np.float64) / HD))
    freqs = np.outer(np.arange(T, dtype=np.float64), inv)  # [T, 64]
    emb = np.concatenate([freqs, freqs], axis=-1)  # [T, 128]
    cos_t = np.cos(emb).T.astype(np.float32).copy()  # [128, T]
    sin_t = np.sin(emb).T.astype(np.float32).copy()
    sin_t[:64, :] *= -1.0  # rotate-half sign folded in (see rope())
    P = np.zeros((128, 128), dtype=np.float32)
    P[:64, 64:] = -np.eye(64, dtype=np.float32)
    P[64:, :64] = np.eye(64, dtype=np.float32)
    rotm = P.T.copy()
    ident = np.eye(128, dtype=np.float32)
    ones = np.ones((128, 128), dtype=np.float32)
    _cache["consts"] = (cos_t, sin_t, rotm, ident, ones)
    return _cache["consts"]


def _in_maps(x, wq, wk, wv, wo):
    cos_t, sin_t, rotm, ident, ones = _host_consts()
    maps = []
    for c in range(NCORES):
        b, g = c // KV, c % KV
        xt = np.ascontiguousarray(
            x[b].reshape(T, 16, 128).transpose(2, 1, 0)).astype(np.float32)
        wq_g = wq[g * NR * HD:(g + 1) * NR * HD]  # [512, D]
        # per-head contiguous slices: wqt{j}[p, dc, jc] = wq_g[j*128+jc, dc*128+p]
        wq_h = wq_g.reshape(NR, HD, 16, 128).transpose(0, 3, 2, 1)  # [j, p, dc, jc]
        wk_g = wk[g * HD:(g + 1) * HD]
        wkt = np.ascontiguousarray(wk_g.reshape(HD, 16, 128).transpose(2, 1, 0))
        wv_g = wv[g * HD:(g + 1) * HD]
        wvt = np.ascontiguousarray(wv_g.reshape(HD, 16, 128).transpose(2, 1, 0))
        wo_g = wo[:, g * NR * HD:(g + 1) * NR * HD]  # [D, 512]
        wot = np.ascontiguousarray(
            wo_g.reshape(D, NR, 128).transpose(2, 1, 0)).astype(ml_dtypes.bfloat16)
        m = {
            "xt": xt, "wkt": wkt.astype(np.float32),
            "wvt": wvt.astype(np.float32), "wot": wot,
            "cosa": cos_t, "sina": sin_t, "rotm": rotm,
            "ident": ident, "ones": ones,
        }
        for j in range(NR):
            m[f"wqt{j}"] = np.ascontiguousarray(wq_h[j]).astype(np.float32)
        maps.append(m)
    return maps


def run_spmd(x, wq, wk, wv, wo, **kw):
    nc = _get_nc()
    maps = _in_maps(x, wq, wk, wv, wo)
    return run_bass_kernel_spmd(nc, maps, core_ids=list(range(NCORES)), **kw)


def kernel(x, wq, wk, wv, wo):
    res = run_spmd(x, wq, wk, wv, wo)
    out = np.zeros((B, T, D), dtype=np.float32)
    for c in range(NCORES):
        out[c // KV] += res.results[c]["out"]
    return out



# revision 2
# speedup vs baseline: 1.1969x; 1.1969x over previous
"""GQA attention kernel for 8 TRN2 NeuronCores.

Problem: B=2, T=2048, D=2048, H=16 q-heads, KV=4 kv-heads, HD=128, RoPE,
non-causal softmax, out projection. f32 reference.

Sharding: 8 cores = 2 batches x 4 kv-groups. Core c handles batch c//4 and
kv-group c%4 (4 q heads + 1 kv head). Each core computes a partial output
x @ wq_g -> attention -> (heads g) @ wo_g^T: full [T, D] partial summed on
host over the 4 groups of each batch (tensor-parallel unshard).

On-device layout: everything transposed ([hd, t], hd=128=partition dim).
All matmul operands are bf16 (fp32 PSUM accumulate): bf16 enables the PE's
fast-weight-load path so the per-matmul LDWEIGHTS (which at fp32r runs
LONGER than the 512-col matmul itself) hides under the streaming, and it
halves DMA + DVE traffic. Measured rel err of the all-bf16 pipeline vs
fp32 reference is ~8e-3 (threshold 2e-2).

 - host feeds xT, wqT, wkT, wvT (d-on-partition chunks) bf16; projections are
   plain lhsT.T @ rhs matmuls with K=d contraction.
 - scores computed transposed: ST[s, t] = k^T q per s-chunk; softmax over s
   (partitions) uses exp on ACT + bf16 chunk-adds on DVE + a ones-matmul
   partition-reduce-broadcast on PE; normalization folded into the OT evac.
 - phase 2 is one flat software pipeline over all 16 (tt, head) pairs: the
   ST stream runs one s-chunk ahead of PV *across head boundaries*, with
   softmax epilogues and out-projection pieces drained as PE filler.
 - out projection: out[t, d] = sum_h OTn_h[j, t]^T wogT[j, d] (bf16), DMA'd
   out as bf16 and summed across kv-group cores on host in fp32.
"""
import os
import sys

for _p in ("/opt/trn_rl_repo", "/root/.axon_site/_ro/trn_rl_repo"):
    if os.path.isdir(_p) and _p not in sys.path:
        sys.path.append(_p)

import numpy as np
import ml_dtypes

import concourse.bass as bass
import concourse.tile as tile
from concourse.tile import add_dep_helper
from concourse import bacc, mybir
from concourse import bass_utils
from concourse.bass_utils import run_bass_kernel_spmd

# If a caller enables tracing (BASS_TRACE=1), artifact upload may have no
# bucket access in this container; fall back to the local dir.
_orig_upload = bass_utils.upload_artifacts


def _safe_upload(tmpdir):
    try:
        return _orig_upload(tmpdir)
    except Exception:
        return tmpdir


bass_utils.upload_artifacts = _safe_upload

B, T, D = 2, 2048, 2048
H, KV, HD = 16, 4, 128
NR = H // KV  # 4 q heads per kv group
NCORES = 8
ROPE_BASE = 10000.0
SCALE = float(HD) ** -0.5

F32R = mybir.dt.float32r
F32 = mybir.dt.float32
BF16 = mybir.dt.bfloat16

_cache = {}


def _build_nc():
    nc = bacc.Bacc("TRN2", target_bir_lowering=False, debug=False,
                   num_devices=NCORES)

    xt_e = nc.dram_tensor("xt", [128, 16, T], BF16, kind="ExternalInput").ap()
    wqt_e = [nc.dram_tensor(f"wqt{j}", [128, 16, HD], BF16,
                            kind="ExternalInput").ap() for j in range(NR)]
    wkt_e = nc.dram_tensor("wkt", [128, 16, HD], BF16, kind="ExternalInput").ap()
    wvt_e = nc.dram_tensor("wvt", [128, 16, HD], BF16, kind="ExternalInput").ap()
    wot_e = nc.dram_tensor("wot", [128, NR, D], BF16, kind="ExternalInput").ap()
    cos_e = nc.dram_tensor("cosa", [128, T], F32R, kind="ExternalInput").ap()
    sin_e = nc.dram_tensor("sina", [128, T], F32R, kind="ExternalInput").ap()
    ident_e = nc.dram_tensor("ident", [128, 128], BF16, kind="ExternalInput").ap()
    ones_e = nc.dram_tensor("ones", [128, 128], BF16, kind="ExternalInput").ap()
    out_e = nc.dram_tensor("out", [T, D], BF16, kind="ExternalOutput").ap()

    with tile.TileContext(nc) as tc:
        import contextlib
        with contextlib.ExitStack() as ctx:
            consts = ctx.enter_context(tc.tile_pool(name="consts", bufs=1))
            weights = ctx.enter_context(tc.tile_pool(name="weights", bufs=1))
            acts = ctx.enter_context(tc.tile_pool(name="acts", bufs=1))

            cos_sb = consts.tile([128, T], F32R, tag="cos")
            sin_sb = consts.tile([128, T], F32R, tag="sin")
            ident_sb = consts.tile([128, 128], BF16, tag="ident")
            ones_sb = consts.tile([128, 128], BF16, tag="ones")
            scratch_sb = consts.tile([128, 2], F32R, tag="scratch")
            wkt_sb = weights.tile([128, 16, HD], BF16, tag="wkt")
            wvt_sb = weights.tile([128, 16, HD], BF16, tag="wvt")
            wqt_sb = [weights.tile([128, 16, HD], BF16, tag=f"wqt{j}",
                                   name=f"wqt{j}_sb") for j in range(NR)]
            wot_sb = weights.tile([128, NR, D], BF16, tag="wot")
            # DMA ordering: sync queue feeds the PE-critical path (wkt then
            # xt tiles); scalar queue gets small consts; gpsimd SWDGE the
            # bulk weights, gated behind the startup matmuls.
            nc.sync.dma_start(out=wkt_sb[:, :4, :], in_=wkt_e[:, :4, :])
            nc.sync.dma_start(out=wkt_sb[:, 4:, :], in_=wkt_e[:, 4:, :])
            nc.scalar.dma_start(out=ident_sb, in_=ident_e)
            nc.scalar.dma_start(out=ones_sb, in_=ones_e)
            dc0 = nc.scalar.dma_start(out=cos_sb[:, :512], in_=cos_e[:, :512])
            ds0 = nc.scalar.dma_start(out=sin_sb[:, :512], in_=sin_e[:, :512])
            # trigger the ACT exp table load (~2.7us) during phase 1 so the
            # first real exp doesn't pay it
            nc.scalar.activation(scratch_sb, cos_sb[:, 0:2],
                                 mybir.ActivationFunctionType.Exp, scale=1.0)
            dwv = nc.gpsimd.dma_start(out=wvt_sb, in_=wvt_e)
            gated_dmas = [(dc0.ins, 1), (ds0.ins, 1)]
            for j in range(NR):
                dq = nc.gpsimd.dma_start(out=wqt_sb[j], in_=wqt_e[j])
                gated_dmas.append((dq.ins, max(0, 3 * j - 1)))
            dcr = nc.scalar.dma_start(out=cos_sb[:, 512:], in_=cos_e[:, 512:])
            gated_dmas.append((dcr.ins, 12))
            dsr = nc.scalar.dma_start(out=sin_sb[:, 512:], in_=sin_e[:, 512:])
            gated_dmas.append((dsr.ins, 12))

            qtr = [acts.tile([128, T], BF16, tag=f"qtr{j}", name=f"qtr{j}") for j in range(NR)]
            ktr = acts.tile([128, T], BF16, tag="ktr")
            v_sb = acts.tile([128, 16, HD], BF16, tag="vsb")  # v natural, s-chunked

            # ---------------- Phase 1: projections + RoPE + v transpose ----
            with tc.tile_pool(name="xt", bufs=8) as xt_pool, \
                 tc.tile_pool(name="rope", bufs=2) as rope_pool, \
                 tc.tile_pool(name="p1ps", bufs=1, space="PSUM") as p1ps, \
                 tc.tile_pool(name="rotps", bufs=2, space="PSUM") as rotps:
                anchors = []  # tt0 k-proj matmul instructions
                for tt in range(4):
                    tsl = slice(tt * 512, (tt + 1) * 512)
                    xq = []
                    for i in range(4):
                        xti = xt_pool.tile([128, 4, 512], BF16, tag="xt")
                        lo = i * 4
                        if tt == 0 and i == 0:
                            # split so the first k matmul waits on 128KB only
                            nc.sync.dma_start(out=xti[:, 0:1, :],
                                              in_=xt_e[:, 0:1, tsl])
                            nc.scalar.dma_start(out=xti[:, 1:4, :],
                                                in_=xt_e[:, 1:4, tsl])
                        else:
                            # stripe across both HWDGE queues for 2x delivery
                            nc.sync.dma_start(out=xti[:, 0:2, :],
                                              in_=xt_e[:, lo:lo + 2, tsl])
                            nc.scalar.dma_start(out=xti[:, 2:4, :],
                                                in_=xt_e[:, lo + 2:lo + 4, tsl])
                        xq.append(xti)
                    qps = [p1ps.tile([128, 512], F32, tag=f"qps{j}", name=f"qps{j}_{tt}") for j in range(NR)]
                    kps = p1ps.tile([128, 512], F32, tag="kps")
                    vps = p1ps.tile([128, 512], F32, tag="vps")
                    for i in range(4):
                        for dc in range(4):
                            g = i * 4 + dc
                            mk = nc.tensor.matmul(kps, wkt_sb[:, g, :],
                                                  xq[i][:, dc, :],
                                                  start=(g == 0), stop=(g == 15))
                            if tt == 0:
                                anchors.append(mk.ins)
                    for i in range(4):
                        for dc in range(4):
                            g = i * 4 + dc
                            nc.tensor.matmul(vps, wvt_sb[:, g, :], xq[i][:, dc, :],
                                             start=(g == 0), stop=(g == 15))
                    for j in range(NR):
                        for i in range(4):
                            for dc in range(4):
                                g = i * 4 + dc
                                nc.tensor.matmul(
                                    qps[j], wqt_sb[j][:, g, :],
                                    xq[i][:, dc, :], start=(g == 0), stop=(g == 15))

                    # RoPE: dst = src*cos + rotate_half(src)*sin, entirely on
                    # DVE via partition-shifted PSUM reads (sign of the lower
                    # half folded into the host sin table).
                    def rope(src, dst):
                        t1 = rope_pool.tile([128, 512], BF16, tag="t1", name="t1")
                        nc.vector.tensor_mul(t1, src, cos_sb[:, tsl])
                        t2 = rope_pool.tile([128, 512], BF16, tag="t2", name="t2")
                        nc.vector.tensor_mul(t2[0:64, :], src[64:128, :],
                                             sin_sb[0:64, tsl])
                        nc.vector.tensor_mul(t2[64:128, :], src[0:64, :],
                                             sin_sb[64:128, tsl])
                        nc.gpsimd.tensor_add(dst, t1, t2)

                    rope(kps, ktr[:, tsl])
                    # v: copy vT psum -> sbuf bf16, PE-transpose 128-blocks
                    vt_sb = rope_pool.tile([128, 512], BF16, tag="vt")
                    nc.scalar.copy(vt_sb, vps)
                    for vb in range(4):
                        tr_ps = rotps.tile([128, 128], BF16, tag="rot")
                        nc.tensor.transpose(tr_ps, vt_sb[:, vb * 128:(vb + 1) * 128],
                                            ident_sb)
                        nc.vector.tensor_copy(v_sb[:, tt * 4 + vb, :], tr_ps)
                    for j in range(NR):
                        rope(qps[j], qtr[j][:, tsl])

                for dins, aidx in gated_dmas:
                    add_dep_helper(dins, anchors[min(aidx, 15)],
                                   reason="gate bulk dma behind startup")

            # ---------------- Phase 2+3: attention + out projection --------
            # One flat software pipeline over 16 heads x 8 s-steps: the ST
            # stream runs one step ahead of PV across head boundaries, so the
            # PE never sits waiting on the last exp of a head. Softmax
            # epilogues and out-projection pieces drain as PE filler.
            dwot = nc.gpsimd.dma_start(out=wot_sb, in_=wot_e)
            add_dep_helper(dwot.ins, anchors[15], reason="gate wot dma")
            with tc.tile_pool(name="p2sb", bufs=3) as p2sb, \
                 tc.tile_pool(name="dens", bufs=3) as dens, \
                 tc.tile_pool(name="otn", bufs=2) as otnp, \
                 tc.tile_pool(name="ostg", bufs=4) as ostg, \
                 tc.tile_pool(name="stps", bufs=2, space="PSUM") as stps, \
                 tc.tile_pool(name="otps", bufs=2, space="PSUM") as otps, \
                 tc.tile_pool(name="outps", bufs=2, space="PSUM") as outps:
                HEADS = [(tt, h) for tt in range(4) for h in range(NR)]
                NG = len(HEADS) * 8  # 128 global pipeline steps
                ex_tiles = {}
                den_tiles = {}
                ot_tiles = {}
                otn_tiles = {}
                epi_q = []    # deferred softmax epilogues
                piece_q = []  # deferred out-projection pieces

                def out_piece(tt, tkc, otn_t, dts):
                    # half a t-chunk of the out projection: 8 matmuls + evacs
                    rows = slice(tt * 512 + tkc * 128, tt * 512 + (tkc + 1) * 128)
                    for dt in dts:
                        o_ps = outps.tile([128, 512], F32, tag="ops",
                                          name=f"o_ps_{tt}_{tkc}_{dt}")
                        for hh in range(NR):
                            nc.tensor.matmul(
                                o_ps, otn_t[:, hh, tkc * 128:(tkc + 1) * 128],
                                wot_sb[:, hh, dt * 512:(dt + 1) * 512],
                                start=(hh == 0), stop=(hh == NR - 1))
                        o_sb = ostg.tile([128, 512], BF16, tag="ostg",
                                         name=f"o_sb_{tt}_{tkc}_{dt}")
                        if dt % 2 == 0:
                            nc.vector.tensor_copy(o_sb, o_ps)
                            nc.sync.dma_start(
                                out=out_e[rows, dt * 512:(dt + 1) * 512], in_=o_sb)
                        else:
                            nc.scalar.copy(o_sb, o_ps)
                            nc.scalar.dma_start(
                                out=out_e[rows, dt * 512:(dt + 1) * 512], in_=o_sb)

                def issue_st(g):
                    hi, s = divmod(g, 8)
                    tt, h = HEADS[hi]
                    tsl = slice(tt * 512, (tt + 1) * 512)
                    st = stps.tile([128, 2, 512], F32, tag="st", name=f"st_{g}")
                    for half in range(2):
                        sc = s * 2 + half
                        nc.tensor.matmul(st[:, half, :],
                                         ktr[:, sc * 128:(sc + 1) * 128],
                                         qtr[h][:, tsl], start=True, stop=True)
                    ex = p2sb.tile([128, 2, 512], BF16, tag="exp",
                                   name=f"ex_{g}")
                    nc.scalar.activation(ex, st,
                                         mybir.ActivationFunctionType.Exp,
                                         scale=SCALE)
                    ex_tiles[g] = ex
                    if s == 0:
                        den = dens.tile([128, 2, 512], BF16, tag="den",
                                        name=f"den_{hi}")
                        den_tiles[hi] = den
                        nc.vector.tensor_copy(den, ex)
                    else:
                        den = den_tiles[hi]
                        nc.vector.tensor_add(den, den, ex)

                def issue_pv(g):
                    hi, s = divmod(g, 8)
                    if s == 0:
                        ot_tiles[hi] = otps.tile([128, 512], F32, tag="ot",
                                                 name=f"ot_{hi}")
                    ot = ot_tiles[hi]
                    ex = ex_tiles.pop(g)
                    for half in range(2):
                        sc = s * 2 + half
                        nc.tensor.matmul(ot, v_sb[:, sc, :], ex[:, half, :],
                                         start=(sc == 0), stop=(sc == 15))
                    if s == 7:
                        epi_q.append(hi)

                def epilogue(hi):
                    tt, h = HEADS[hi]
                    den = den_tiles.pop(hi)
                    ot = ot_tiles.pop(hi)
                    otn_t = otn_tiles[tt]
                    # partition-reduce+broadcast denominator on PE, both
                    # halves accumulated into one PSUM bank
                    bc_ps = outps.tile([128, 512], F32, tag="ops",
                                       name=f"bc_{hi}")
                    nc.tensor.matmul(bc_ps, ones_sb, den[:, 0, :],
                                     start=True, stop=False)
                    nc.tensor.matmul(bc_ps, ones_sb, den[:, 1, :],
                                     start=False, stop=True)
                    rden = dens.tile([128, 512], F32, tag="rden",
                                     name=f"rden_{hi}")
                    nc.vector.reciprocal_approx_fast(rden, bc_ps)
                    nc.vector.tensor_tensor(out=otn_t[:, h, :], in0=ot,
                                            in1=rden,
                                            op=mybir.AluOpType.mult)
                    if h == NR - 1:
                        for tkc in range(4):
                            for dts in ((0, 1), (2, 3)):
                                piece_q.append((tt, tkc, otn_t, dts))

                for g in range(NG + 1):
                    if g < NG:
                        hi, s = divmod(g, 8)
                        tt, h = HEADS[hi]
                        if s == 0 and h == 0:
                            otn_tiles[tt] = otnp.tile([128, NR, 512], BF16,
                                                      tag="otn",
                                                      name=f"otn_{tt}")
                        issue_st(g)
                    if g >= 1:
                        issue_pv(g - 1)
                        s1 = (g - 1) % 8
                        if s1 == 2 and epi_q:
                            epilogue(epi_q.pop(0))
                        if s1 in (4, 6) and piece_q:
                            out_piece(*piece_q.pop(0))
                # tail: last head's epilogue + final t-tile's out projection
                while epi_q:
                    epilogue(epi_q.pop(0))
                while piece_q:
                    out_piece(*piece_q.pop(0))
    nc.compile()
    return nc


def _get_nc():
    if "nc" not in _cache:
        _cache["nc"] = _build_nc()
    return _cache["nc"]


def _host_consts():
    if "consts" in _cache:
        return _cache["consts"]
    inv = 1.0 / (ROPE_BASE ** (np.arange(0, HD, 2, dtype=np.float64) / HD))
    freqs = np.outer(np.arange(T, dtype=np.float64), inv)  # [T, 64]
    emb = np.concatenate([freqs, freqs], axis=-1)  # [T, 128]
    cos_t = np.cos(emb).T.astype(np.float32).copy()  # [128, T]
    sin_t = np.sin(emb).T.astype(np.float32).copy()
    sin_t[:64, :] *= -1.0  # rotate-half sign folded in (see rope())
    ident = np.eye(128, dtype=np.float32).astype(ml_dtypes.bfloat16)
    ones = np.ones((128, 128), dtype=ml_dtypes.bfloat16)
    _cache["consts"] = (cos_t, sin_t, ident, ones)
    return _cache["consts"]


def _in_maps(x, wq, wk, wv, wo):
    cos_t, sin_t, ident, ones = _host_consts()
    bf = ml_dtypes.bfloat16
    maps = []
    for c in range(NCORES):
        b, g = c // KV, c % KV
        xt = np.ascontiguousarray(
            x[b].reshape(T, 16, 128).transpose(2, 1, 0)).astype(bf)
        wq_g = wq[g * NR * HD:(g + 1) * NR * HD]  # [512, D]
        # per-head contiguous slices: wqt{j}[p, dc, jc] = wq_g[j*128+jc, dc*128+p]
        wq_h = wq_g.reshape(NR, HD, 16, 128).transpose(0, 3, 2, 1)  # [j, p, dc, jc]
        wk_g = wk[g * HD:(g + 1) * HD]
        wkt = np.ascontiguousarray(wk_g.reshape(HD, 16, 128).transpose(2, 1, 0))
        wv_g = wv[g * HD:(g + 1) * HD]
        wvt = np.ascontiguousarray(wv_g.reshape(HD, 16, 128).transpose(2, 1, 0))
        wo_g = wo[:, g * NR * HD:(g + 1) * NR * HD]  # [D, 512]
        wot = np.ascontiguousarray(
            wo_g.reshape(D, NR, 128).transpose(2, 1, 0)).astype(bf)
        m = {
            "xt": xt, "wkt": wkt.astype(bf),
            "wvt": wvt.astype(bf), "wot": wot,
            "cosa": cos_t, "sina": sin_t,
            "ident": ident, "ones": ones,
        }
        for j in range(NR):
            m[f"wqt{j}"] = np.ascontiguousarray(wq_h[j]).astype(bf)
        maps.append(m)
    return maps


def run_spmd(x, wq, wk, wv, wo, **kw):
    nc = _get_nc()
    maps = _in_maps(x, wq, wk, wv, wo)
    return run_bass_kernel_spmd(nc, maps, core_ids=list(range(NCORES)), **kw)


def kernel(x, wq, wk, wv, wo):
    res = run_spmd(x, wq, wk, wv, wo)
    out = np.zeros((B, T, D), dtype=np.float32)
    for c in range(NCORES):
        out[c // KV] += res.results[c]["out"].astype(np.float32)
    return out


# revision 3
# speedup vs baseline: 1.2858x; 1.0742x over previous
"""GQA attention kernel for 8 TRN2 NeuronCores.

Problem: B=2, T=2048, D=2048, H=16 q-heads, KV=4 kv-heads, HD=128, RoPE,
non-causal softmax, out projection. f32 reference.

Sharding: 8 cores = 2 batches x 4 kv-groups. Core c handles batch c//4 and
kv-group c%4 (4 q heads + 1 kv head). Each core computes a partial output
x @ wq_g -> attention -> (heads g) @ wo_g^T: full [T, D] partial summed on
host over the 4 groups of each batch (tensor-parallel unshard).

On-device layout: everything transposed ([hd, t], hd=128=partition dim).
All matmul operands are bf16 (fp32 PSUM accumulate): bf16 enables the PE's
fast-weight-load path and halves DMA + DVE traffic. Measured rel err of the
all-bf16 pipeline vs the fp32 reference is ~1e-2 (threshold 2e-2).

Even a fully-overlapped LDWEIGHTS steals SBUF->PE streaming bandwidth from
the moving operand (~43ns per 512-col matmul, measured), so the kernel is
structured to reuse the stationary operand across consecutive matmuls and a
post-compile pass drops the redundant InstLdweights that tile_legalize
emits per matmul:
 - projections run g-chunk-major over tt-pairs: one weight chunk load feeds
   the two t-tiles of the pair (K and V interleaved per g so the x DMA
   stream is consumed at the delivery rate).
 - out-projection pieces run head-major: one otn chunk load feeds two
   512-wide output column tiles.
 - scores computed transposed: ST[s, t] = k^T q per s-chunk; softmax over s
   (partitions) uses exp on ACT + bf16 chunk-adds on DVE + a ones-matmul
   partition-reduce-broadcast on PE; normalization folded into the OT evac.
 - phase 2 is one flat software pipeline over all 16 (tt, head) pairs: the
   ST stream runs one s-chunk ahead of PV across head boundaries, with
   softmax epilogues and out-projection pieces drained as PE filler (a few
   pieces held back to cover the final head's softmax-epilogue latency).
"""
import os
import sys

for _p in ("/opt/trn_rl_repo", "/root/.axon_site/_ro/trn_rl_repo"):
    if os.path.isdir(_p) and _p not in sys.path:
        sys.path.append(_p)

import numpy as np
import ml_dtypes

import concourse.bass as bass
import concourse.tile as tile
from concourse.tile import add_dep_helper
from concourse import bacc, mybir
from concourse import bass_utils
from concourse.bass_utils import run_bass_kernel_spmd

# If a caller enables tracing (BASS_TRACE=1), artifact upload may have no
# bucket access in this container; fall back to the local dir.
_orig_upload = bass_utils.upload_artifacts


def _safe_upload(tmpdir):
    try:
        return _orig_upload(tmpdir)
    except Exception:
        return tmpdir


bass_utils.upload_artifacts = _safe_upload

B, T, D = 2, 2048, 2048
H, KV, HD = 16, 4, 128
NR = H // KV  # 4 q heads per kv group
NCORES = 8
ROPE_BASE = 10000.0
SCALE = float(HD) ** -0.5

F32R = mybir.dt.float32r
F32 = mybir.dt.float32
BF16 = mybir.dt.bfloat16

_cache = {}


def _elide_redundant_ldweights(nc):
    """Drop InstLdweights that reload the weights already resident in the PE
    array (same weights AP as the previous load, no semaphore sync of its
    own). tile_legalize emits one load per matmul; the PE keeps the
    stationary operand across matmuls, so consecutive same-weight matmuls
    only need the first load (validated on hardware)."""
    removed = 0
    for f in nc.m.functions:
        for b in f.blocks:
            insts = b.instructions
            keep, last_key = [], None
            for ins in insts:
                t = type(ins).__name__
                if t == "InstLdweights":
                    key = (str(ins.ins[0]), bool(ins.is_transpose),
                           ins.perf_mode)
                    if key == last_key and ins.sync_info is None:
                        removed += 1
                        continue
                    last_key = key
                elif t == "InstDrain":
                    last_key = None
                keep.append(ins)
            if len(keep) != len(insts):
                insts[:] = keep
    return removed


def _build_nc():
    nc = bacc.Bacc("TRN2", target_bir_lowering=False, debug=False,
                   num_devices=NCORES)

    xt_e = nc.dram_tensor("xt", [128, 16, T], BF16, kind="ExternalInput").ap()
    wqt_e = [nc.dram_tensor(f"wqt{j}", [128, 16, HD], BF16,
                            kind="ExternalInput").ap() for j in range(NR)]
    wkt_e = nc.dram_tensor("wkt", [128, 16, HD], BF16, kind="ExternalInput").ap()
    wvt_e = nc.dram_tensor("wvt", [128, 16, HD], BF16, kind="ExternalInput").ap()
    wot_e = nc.dram_tensor("wot", [128, NR, D], BF16, kind="ExternalInput").ap()
    cos_e = nc.dram_tensor("cosa", [128, T], F32R, kind="ExternalInput").ap()
    sin_e = nc.dram_tensor("sina", [128, T], F32R, kind="ExternalInput").ap()
    ident_e = nc.dram_tensor("ident", [128, 128], BF16, kind="ExternalInput").ap()
    ones_e = nc.dram_tensor("ones", [128, 128], BF16, kind="ExternalInput").ap()
    out_e = nc.dram_tensor("out", [T, D], BF16, kind="ExternalOutput").ap()

    with tile.TileContext(nc) as tc:
        import contextlib
        with contextlib.ExitStack() as ctx:
            consts = ctx.enter_context(tc.tile_pool(name="consts", bufs=1))
            weights = ctx.enter_context(tc.tile_pool(name="weights", bufs=1))
            acts = ctx.enter_context(tc.tile_pool(name="acts", bufs=1))

            cos_sb = consts.tile([128, T], F32R, tag="cos")
            sin_sb = consts.tile([128, T], F32R, tag="sin")
            ident_sb = consts.tile([128, 128], BF16, tag="ident")
            ones_sb = consts.tile([128, 128], BF16, tag="ones")
            scratch_sb = consts.tile([128, 2], F32R, tag="scratch")
            wkt_sb = weights.tile([128, 16, HD], BF16, tag="wkt")
            wvt_sb = weights.tile([128, 16, HD], BF16, tag="wvt")
            wqt_sb = [weights.tile([128, 16, HD], BF16, tag=f"wqt{j}",
                                   name=f"wqt{j}_sb") for j in range(NR)]
            wot_sb = weights.tile([128, NR, D], BF16, tag="wot")
            x_sb = acts.tile([128, 16, T], BF16, tag="xsb")  # full x, resident

            # --- startup DMA schedule ---------------------------------------
            # sync queue: the PE-critical path: first K/V weight chunks, then
            # x first-halves (even g). scalar queue: ident (tiny, unblocks the
            # ACT exp-table preload + transposes), x first-halves (odd g).
            # gpsimd SWDGE: remaining weights; gated bulk: cos/sin/ones/wq/wo
            # and x second-halves.
            nc.sync.dma_start(out=wkt_sb[:, 0:2, :], in_=wkt_e[:, 0:2, :])
            nc.sync.dma_start(out=wvt_sb[:, 0:2, :], in_=wvt_e[:, 0:2, :])
            nc.scalar.dma_start(out=ident_sb, in_=ident_e)
            # trigger the ACT exp table load (~2.7us) during phase 1 so the
            # first real exp doesn't pay it
            nc.scalar.activation(scratch_sb, ident_sb[:, 0:2],
                                 mybir.ActivationFunctionType.Exp, scale=1.0)
            nc.gpsimd.dma_start(out=wkt_sb[:, 2:, :], in_=wkt_e[:, 2:, :])
            nc.gpsimd.dma_start(out=wvt_sb[:, 2:, :], in_=wvt_e[:, 2:, :])
            # x first halves (t in [0, 1024)) g-ascending, striped by parity
            for g in range(16):
                q = nc.sync if g % 2 == 0 else nc.scalar
                q.dma_start(out=x_sb[:, g, 0:1024], in_=xt_e[:, g, 0:1024])
            gated_dmas = []  # (dma ins, anchor idx)
            dc0 = nc.scalar.dma_start(out=cos_sb[:, :1024], in_=cos_e[:, :1024])
            ds0 = nc.scalar.dma_start(out=sin_sb[:, :1024], in_=sin_e[:, :1024])
            don = nc.scalar.dma_start(out=ones_sb, in_=ones_e)
            gated_dmas += [(dc0.ins, 8), (ds0.ins, 8), (don.ins, 8)]
            # x second halves (t in [1024, 2048)), needed from the second
            # K+V pair-pass (~17us in)
            for g in range(16):
                q = nc.sync if g % 2 == 0 else nc.scalar
                dx = q.dma_start(out=x_sb[:, g, 1024:2048],
                                 in_=xt_e[:, g, 1024:2048])
                gated_dmas.append((dx.ins, 4 + (g // 4) * 8))
            for j in range(NR):
                dq = nc.gpsimd.dma_start(out=wqt_sb[j], in_=wqt_e[j])
                gated_dmas.append((dq.ins, 8 + 8 * j))
            dcr = nc.scalar.dma_start(out=cos_sb[:, 1024:], in_=cos_e[:, 1024:])
            dsr = nc.scalar.dma_start(out=sin_sb[:, 1024:], in_=sin_e[:, 1024:])
            gated_dmas += [(dcr.ins, 40), (dsr.ins, 40)]

            qtr = [acts.tile([128, T], BF16, tag=f"qtr{j}", name=f"qtr{j}") for j in range(NR)]
            ktr = acts.tile([128, T], BF16, tag="ktr")
            v_sb = acts.tile([128, 16, HD], BF16, tag="vsb")  # v natural, s-chunked

            # ---------------- Phase 1: projections + RoPE + v transpose ----
            # g-chunk-major over tt-pairs so each weight-chunk load feeds two
            # matmuls (the elision pass drops the second load).
            PAIRS = ((0, 1), (2, 3))
            with tc.tile_pool(name="rope", bufs=2) as rope_pool, \
                 tc.tile_pool(name="p1ps", bufs=2, space="PSUM") as p1ps, \
                 tc.tile_pool(name="vps", bufs=1, space="PSUM") as vpsp, \
                 tc.tile_pool(name="rotps", bufs=2, space="PSUM") as rotps:
                anchors = []  # first K+V pass matmul instructions

                def rope(src, dst, tsl):
                    # dst = src*cos + rotate_half(src)*sin on DVE via
                    # partition-shifted PSUM reads (lower-half sign folded
                    # into the host sin table); final add on gpsimd.
                    t1 = rope_pool.tile([128, 512], BF16, tag="t1", name="t1")
                    nc.vector.tensor_mul(t1, src, cos_sb[:, tsl])
                    t2 = rope_pool.tile([128, 512], BF16, tag="t2", name="t2")
                    nc.vector.tensor_mul(t2[0:64, :], src[64:128, :],
                                         sin_sb[0:64, tsl])
                    nc.vector.tensor_mul(t2[64:128, :], src[0:64, :],
                                         sin_sb[64:128, tsl])
                    nc.gpsimd.tensor_add(dst, t1, t2)

                # K+V passes, one per tt-pair
                for pi, pair in enumerate(PAIRS):
                    kps = p1ps.tile([128, 2, 512], F32, tag="kps",
                                    name=f"kps_{pi}")
                    vps = vpsp.tile([128, 2, 512], F32, tag="vps",
                                    name=f"vps_{pi}")
                    for g in range(16):
                        for ti, tt in enumerate(pair):
                            tsl = slice(tt * 512, (tt + 1) * 512)
                            mk = nc.tensor.matmul(kps[:, ti, :],
                                                  wkt_sb[:, g, :],
                                                  x_sb[:, g, tsl],
                                                  start=(g == 0), stop=(g == 15))
                            if pi == 0:
                                anchors.append(mk.ins)
                        for ti, tt in enumerate(pair):
                            tsl = slice(tt * 512, (tt + 1) * 512)
                            mv = nc.tensor.matmul(vps[:, ti, :],
                                                  wvt_sb[:, g, :],
                                                  x_sb[:, g, tsl],
                                                  start=(g == 0), stop=(g == 15))
                            if pi == 0:
                                anchors.append(mv.ins)
                    for ti, tt in enumerate(pair):
                        tsl = slice(tt * 512, (tt + 1) * 512)
                        rope(kps[:, ti, :], ktr[:, tsl], tsl)
                        # v: copy vT psum -> sbuf bf16, PE-transpose 128-blocks
                        vt_sb = rope_pool.tile([128, 512], BF16, tag="vt",
                                               name=f"vt_{tt}")
                        nc.scalar.copy(vt_sb, vps[:, ti, :])
                        for vb in range(4):
                            tr_ps = rotps.tile([128, 128], BF16, tag="rot")
                            nc.tensor.transpose(
                                tr_ps, vt_sb[:, vb * 128:(vb + 1) * 128],
                                ident_sb)
                            nc.vector.tensor_copy(v_sb[:, tt * 4 + vb, :], tr_ps)

                # Q passes: per head, per tt-pair
                for j in range(NR):
                    for pair in PAIRS:
                        qps = p1ps.tile([128, 2, 512], F32, tag="kps",
                                        name=f"qps_{j}_{pair[0]}")
                        for g in range(16):
                            for ti, tt in enumerate(pair):
                                tsl = slice(tt * 512, (tt + 1) * 512)
                                nc.tensor.matmul(qps[:, ti, :],
                                                 wqt_sb[j][:, g, :],
                                                 x_sb[:, g, tsl],
                                                 start=(g == 0), stop=(g == 15))
                        for ti, tt in enumerate(pair):
                            tsl = slice(tt * 512, (tt + 1) * 512)
                            rope(qps[:, ti, :], qtr[j][:, tsl], tsl)

                for dins, aidx in gated_dmas:
                    add_dep_helper(dins, anchors[min(aidx, len(anchors) - 1)],
                                   reason="gate bulk dma behind startup")

            # ---------------- Phase 2+3: attention + out projection --------
            # One flat software pipeline over 16 heads x 8 s-steps: the ST
            # stream runs one step ahead of PV across head boundaries, so the
            # PE never sits waiting on the last exp of a head. Softmax
            # epilogues and out-projection pieces drain as PE filler.
            dwot = nc.gpsimd.dma_start(out=wot_sb, in_=wot_e)
            add_dep_helper(dwot.ins, anchors[-1], reason="gate wot dma")
            with tc.tile_pool(name="p2sb", bufs=3) as p2sb, \
                 tc.tile_pool(name="dens", bufs=3) as dens, \
                 tc.tile_pool(name="otn", bufs=2) as otnp, \
                 tc.tile_pool(name="ostg", bufs=4) as ostg, \
                 tc.tile_pool(name="stps", bufs=2, space="PSUM") as stps, \
                 tc.tile_pool(name="otps", bufs=2, space="PSUM") as otps, \
                 tc.tile_pool(name="outps", bufs=2, space="PSUM") as outps:
                HEADS = [(tt, h) for tt in range(4) for h in range(NR)]
                NG = len(HEADS) * 8  # 128 global pipeline steps
                ex_tiles = {}
                den_tiles = {}
                ot_tiles = {}
                otn_tiles = {}
                epi_q = []    # deferred softmax epilogues
                piece_q = []  # deferred out-projection pieces

                def out_piece(tt, tkc, otn_t, dts):
                    # half a t-chunk of the out projection, head-major so the
                    # otn chunk weight load is shared by the two column tiles
                    rows = slice(tt * 512 + tkc * 128, tt * 512 + (tkc + 1) * 128)
                    o_ps = {dt: outps.tile([128, 512], F32, tag="ops",
                                           name=f"o_ps_{tt}_{tkc}_{dt}")
                            for dt in dts}
                    for hh in range(NR):
                        for di, dt in enumerate(dts):
                            nc.tensor.matmul(
                                o_ps[dt], otn_t[:, hh, tkc * 128:(tkc + 1) * 128],
                                wot_sb[:, hh, dt * 512:(dt + 1) * 512],
                                start=(hh == 0), stop=(hh == NR - 1))
                    for dt in dts:
                        o_sb = ostg.tile([128, 512], BF16, tag="ostg",
                                         name=f"o_sb_{tt}_{tkc}_{dt}")
                        if dt % 2 == 0:
                            nc.vector.tensor_copy(o_sb, o_ps[dt])
                            nc.sync.dma_start(
                                out=out_e[rows, dt * 512:(dt + 1) * 512], in_=o_sb)
                        else:
                            nc.scalar.copy(o_sb, o_ps[dt])
                            nc.scalar.dma_start(
                                out=out_e[rows, dt * 512:(dt + 1) * 512], in_=o_sb)

                def issue_st(g):
                    hi, s = divmod(g, 8)
                    tt, h = HEADS[hi]
                    tsl = slice(tt * 512, (tt + 1) * 512)
                    st = stps.tile([128, 2, 512], F32, tag="st", name=f"st_{g}")
                    for half in range(2):
                        sc = s * 2 + half
                        nc.tensor.matmul(st[:, half, :],
                                         ktr[:, sc * 128:(sc + 1) * 128],
                                         qtr[h][:, tsl], start=True, stop=True)
                    ex = p2sb.tile([128, 2, 512], BF16, tag="exp",
                                   name=f"ex_{g}")
                    nc.scalar.activation(ex, st,
                                         mybir.ActivationFunctionType.Exp,
                                         scale=SCALE)
                    ex_tiles[g] = ex
                    if s == 0:
                        den = dens.tile([128, 2, 512], BF16, tag="den",
                                        name=f"den_{hi}")
                        den_tiles[hi] = den
                        nc.vector.tensor_copy(den, ex)
                    else:
                        den = den_tiles[hi]
                        nc.vector.tensor_add(den, den, ex)

                def issue_pv(g):
                    hi, s = divmod(g, 8)
                    if s == 0:
                        ot_tiles[hi] = otps.tile([128, 512], F32, tag="ot",
                                                 name=f"ot_{hi}")
                    ot = ot_tiles[hi]
                    ex = ex_tiles.pop(g)
                    for half in range(2):
                        sc = s * 2 + half
                        nc.tensor.matmul(ot, v_sb[:, sc, :], ex[:, half, :],
                                         start=(sc == 0), stop=(sc == 15))
                    if s == 7:
                        epi_q.append(hi)

                def epilogue(hi):
                    tt, h = HEADS[hi]
                    den = den_tiles.pop(hi)
                    ot = ot_tiles.pop(hi)
                    otn_t = otn_tiles[tt]
                    # partition-reduce+broadcast denominator on PE, both
                    # halves accumulated into one PSUM bank (one ones load)
                    bc_ps = outps.tile([128, 512], F32, tag="ops",
                                       name=f"bc_{hi}")
                    nc.tensor.matmul(bc_ps, ones_sb, den[:, 0, :],
                                     start=True, stop=False)
                    nc.tensor.matmul(bc_ps, ones_sb, den[:, 1, :],
                                     start=False, stop=True)
                    rden = dens.tile([128, 512], F32, tag="rden",
                                     name=f"rden_{hi}")
                    nc.vector.reciprocal_approx_fast(rden, bc_ps)
                    nc.vector.tensor_tensor(out=otn_t[:, h, :], in0=ot,
                                            in1=rden,
                                            op=mybir.AluOpType.mult)
                    if h == NR - 1:
                        for tkc in range(4):
                            for dts in ((0, 1), (2, 3)):
                                piece_q.append((tt, tkc, otn_t, dts))

                for g in range(NG + 1):
                    if g < NG:
                        hi, s = divmod(g, 8)
                        tt, h = HEADS[hi]
                        if s == 0 and h == 0:
                            otn_tiles[tt] = otnp.tile([128, NR, 512], BF16,
                                                      tag="otn",
                                                      name=f"otn_{tt}")
                        issue_st(g)
                    if g >= 1:
                        issue_pv(g - 1)
                        s1 = (g - 1) % 8
                        if s1 == 1 and epi_q:
                            epilogue(epi_q.pop(0))
                        # hold a few pieces back to fill the final head's
                        # epilogue-latency bubble at stream end
                        if s1 in (3, 5) and len(piece_q) > 3:
                            out_piece(*piece_q.pop(0))
                # tail: last head's epilogue + remaining out projection
                while epi_q:
                    epilogue(epi_q.pop(0))
                while piece_q:
                    out_piece(*piece_q.pop(0))
    nc.compile()
    n = _elide_redundant_ldweights(nc)
    sys.stderr.write(f"kernel: elided {n} redundant LDWEIGHTS\n")
    return nc


def _get_nc():
    if "nc" not in _cache:
        _cache["nc"] = _build_nc()
    return _cache["nc"]


def _host_consts():
    if "consts" in _cache:
        return _cache["consts"]
    inv = 1.0 / (ROPE_BASE ** (np.arange(0, HD, 2, dtype=np.float64) / HD))
    freqs = np.outer(np.arange(T, dtype=np.float64), inv)  # [T, 64]
    emb = np.concatenate([freqs, freqs], axis=-1)  # [T, 128]
    cos_t = np.cos(emb).T.astype(np.float32).copy()  # [128, T]
    sin_t = np.sin(emb).T.astype(np.float32).copy()
    sin_t[:64, :] *= -1.0  # rotate-half sign folded in (see rope())
    ident = np.eye(128, dtype=np.float32).astype(ml_dtypes.bfloat16)
    ones = np.ones((128, 128), dtype=ml_dtypes.bfloat16)
    _cache["consts"] = (cos_t, sin_t, ident, ones)
    return _cache["consts"]


def _in_maps(x, wq, wk, wv, wo):
    cos_t, sin_t, ident, ones = _host_consts()
    bf = ml_dtypes.bfloat16
    maps = []
    for c in range(NCORES):
        b, g = c // KV, c % KV
        xt = np.ascontiguousarray(
            x[b].reshape(T, 16, 128).transpose(2, 1, 0)).astype(bf)
        wq_g = wq[g * NR * HD:(g + 1) * NR * HD]  # [512, D]
        # per-head contiguous slices: wqt{j}[p, dc, jc] = wq_g[j*128+jc, dc*128+p]
        wq_h = wq_g.reshape(NR, HD, 16, 128).transpose(0, 3, 2, 1)  # [j, p, dc, jc]
        wk_g = wk[g * HD:(g + 1) * HD]
        wkt = np.ascontiguousarray(wk_g.reshape(HD, 16, 128).transpose(2, 1, 0))
        wv_g = wv[g * HD:(g + 1) * HD]
        wvt = np.ascontiguousarray(wv_g.reshape(HD, 16, 128).transpose(2, 1, 0))
        wo_g = wo[:, g * NR * HD:(g + 1) * NR * HD]  # [D, 512]
        wot = np.ascontiguousarray(
            wo_g.reshape(D, NR, 128).transpose(2, 1, 0)).astype(bf)
        m = {
            "xt": xt, "wkt": wkt.astype(bf),
            "wvt": wvt.astype(bf), "wot": wot,
            "cosa": cos_t, "sina": sin_t,
            "ident": ident, "ones": ones,
        }
        for j in range(NR):
            m[f"wqt{j}"] = np.ascontiguousarray(wq_h[j]).astype(bf)
        maps.append(m)
    return maps


def run_spmd(x, wq, wk, wv, wo, **kw):
    nc = _get_nc()
    maps = _in_maps(x, wq, wk, wv, wo)
    return run_bass_kernel_spmd(nc, maps, core_ids=list(range(NCORES)), **kw)


def kernel(x, wq, wk, wv, wo):
    res = run_spmd(x, wq, wk, wv, wo)
    out = np.zeros((B, T, D), dtype=np.float32)
    for c in range(NCORES):
        out[c // KV] += res.results[c]["out"].astype(np.float32)
    return out
